# revision 23
# baseline (speedup 1.0000x reference)
"""DisenGCN Trainium2 kernel (8 NeuronCores, SPMD node-parallel).

Strategy (hardcoded from the problem spec):
  - Shard the 20000 nodes across 8 cores (2500/core, padded to 2560 = 20*128).
  - Weights replicated; per layer each core computes its local normalized
    embedding shard, AllGathers the full table to DRAM, then dma_gathers its
    2560*16 neighbor rows into SBUF and runs the 5 capsule-routing iterations
    on the Vector/Scalar engines (node-major layout: nodes on partitions).
  - Features are stored in (d, k) transposed capsule order (host-side weight
    permutation) so per-capsule reductions are flat prefix-halving tree adds
    and all broadcast multiplies have contiguous innermost APs (DVE 2x mode).
  - Nodes are processed in quarters so the dma_gather descriptor generation
    (GPSIMD-bound) of quarter q+1 overlaps the routing DVE work of quarter q.
  - fp16 storage/compute; fp32 for softmax denominators / norms.
"""

import os
import sys
import numpy as np

for _p in ("/opt/trn_rl_repo", "/root/.axon_site/_ro/trn_rl_repo"):
    if os.path.isdir(_p) and _p not in sys.path:
        sys.path.insert(0, _p)

import concourse.bass as bass  # noqa: E402
import concourse.tile as tile  # noqa: E402
from concourse import bacc, mybir  # noqa: E402
from concourse.bass_utils import run_bass_kernel_spmd  # noqa: E402
from concourse.library_config import mlp as mlp_lib  # noqa: E402
from concourse.masks import make_identity  # noqa: E402

FP16 = mybir.dt.float16
FP32 = mybir.dt.float32
I16 = mybir.dt.int16

N_CORES = 8
M = 16          # neighbor fanout
KD = 16         # per-capsule dim
CAPS = [8, 7, 6, 5, 4, 3]
ROUTIT = 5
GCH = 1024      # dma_gather rows per instruction (>1024 crashes the exec unit)


def perm_dk(k):
    """new position d*k+kk  <-  old feature index kk*KD+d."""
    p = np.empty(k * KD, np.int64)
    for d in range(KD):
        for kk in range(k):
            p[d * k + kk] = kk * KD + d
    return p


class Cfg:
    def __init__(self, nshard, feat, n_cores=N_CORES, caps=CAPS, routit=ROUTIT,
                 qsplit=None):
        self.n_cores = n_cores
        self.nshard = nshard
        self.np_ = ((nshard + 127) // 128) * 128
        self.nt = self.np_ // 128
        self.E = self.np_ * M
        self.feat = feat
        self.fpad = ((feat + 127) // 128) * 128
        self.fchunks = self.fpad // 128
        self.caps = caps
        self.routit = routit
        self.kmax = max(caps)
        self.ntab = n_cores * self.np_
        if qsplit is None:
            qsplit = 4 if self.nt % 4 == 0 else 1
        self.qs = qsplit                    # node quarters
        self.nth = self.nt // qsplit        # node tiles per quarter


FULL_CFG = Cfg(2500, 500)


def build_nc(cfg: Cfg):
    nc = bacc.Bacc("TRN2", target_bir_lowering=False, debug=False,
                   num_devices=cfg.n_cores)
    NT, NP, E, QS, NTh = cfg.nt, cfg.np_, cfg.E, cfg.qs, cfg.nth
    KM = cfg.kmax

    feat_t = nc.dram_tensor("feat_t", [cfg.fpad, NP], FP16, kind="ExternalInput")
    pca_wT = nc.dram_tensor("pca_wT", [cfg.fpad, 128], FP16, kind="ExternalInput")
    pca_b = nc.dram_tensor("pca_b", [128, 1], FP32, kind="ExternalInput")
    wTs, bs = [], []
    for i in range(1, len(cfg.caps)):
        fo = cfg.caps[i] * KD
        wTs.append(nc.dram_tensor(f"w{i}T", [128, fo], FP16, kind="ExternalInput"))
        bs.append(nc.dram_tensor(f"b{i}", [128, 1], FP32, kind="ExternalInput"))
    gidx_d = nc.dram_tensor("gidx", [128, E // 16], I16, kind="ExternalInput")
    outs_d = []
    fdims = [128] + [k * KD for k in cfg.caps]
    for li, f in enumerate(fdims):
        outs_d.append(nc.dram_tensor(f"y{li}", [NP, f], FP16, kind="ExternalOutput"))

    tshard = nc.dram_tensor("tshard", [NP, 128], FP16)
    table = nc.dram_tensor("table", [cfg.ntab, 128], FP16, addr_space="Shared")

    def sb(name, shape, dt):
        return nc.alloc_sbuf_tensor(name, shape, dt).ap()

    Z = sb("Z", [128, M * NT, 128], FP16)       # [q][m][j] block order
    XC = sb("XC", [128, NT, 128], FP16)
    XN = sb("XN", [128, NT, 128], FP16)
    U = sb("U", [128, NT, 128], FP16)
    U2 = sb("U2", [128, NT, 128], FP16)
    XT = sb("XT", [128, NP], FP16)
    XLT = sb("XLT", [128, NP], FP16)
    PR = sb("PR", [128, M * NTh, 128], FP16)    # per-quarter scratch
    S = sb("S", [128, M * NTh * KM], FP16)      # scores -> exp -> p (in place)
    DEN = sb("DEN", [128, M * NTh], FP32)
    RIV = sb("RIV", [128, M * NTh], FP32)
    SQ = sb("SQ", [128, NTh, 128], FP32)        # normalize scratch
    RIN = sb("RIN", [128, NTh * KM], FP32)
    RIN2 = sb("RIN2", [128, NTh * KM], FP32)
    GIDX = sb("GIDX", [128, E // 16], I16)
    IDT = sb("IDT", [128, 128], FP16)
    FT = sb("FT", [128, cfg.fchunks, NP], FP16)
    PW = sb("PW", [128, cfg.fchunks, 128], FP16)
    PB = sb("PB", [128, 1], FP32)
    WTS = [sb(f"WTS{i}", [128, cfg.caps[i] * KD], FP16)
           for i in range(1, len(cfg.caps))]
    BS = [sb(f"BS{i}", [128, 1], FP32) for i in range(1, len(cfg.caps))]

    Zb = Z.rearrange("p (b f) -> p b f", f=128) if Z.ndim == 2 else Z
    Z5 = Z.rearrange("p (q m j) f -> p q m j f", q=QS, m=M)

    with tile.TileContext(nc) as tc:
        import contextlib
        ctx = contextlib.ExitStack()
        with ctx:
            psum = ctx.enter_context(tc.tile_pool(name="psum", bufs=4, space="PSUM"))
            nc.gpsimd.load_library(mlp_lib)
            make_identity(nc, IDT)

            nc.sync.dma_start(GIDX, gidx_d.ap())
            nc.sync.dma_start(FT, feat_t.ap().rearrange("(c p) n -> p c n", p=128))
            nc.sync.dma_start(PW, pca_wT.ap().rearrange("(c p) f -> p c f", p=128))
            nc.sync.dma_start(PB, pca_b.ap())
            for i in range(len(cfg.caps) - 1):
                nc.sync.dma_start(WTS[i], wTs[i].ap())
                nc.sync.dma_start(BS[i], bs[i].ap())

            def transpose_block(dst_ap, src_ap, fin, fout):
                pt = psum.tile([128, 128], FP16, tag="pt")
                nc.tensor.transpose(pt[:fout, :fin], src_ap, IDT[:fin, :fin])
                nc.scalar.copy(dst_ap, pt[:fout, :fin])

            # ---- PCA: XLT = relu(pca_w @ feat + b), then node-major XC ----
            nchunk = max(1, NP // 512)
            csz = NP // nchunk
            for c in range(nchunk):
                pl = psum.tile([128, csz], FP32, tag="pl")
                for q in range(cfg.fchunks):
                    nc.tensor.matmul(
                        pl[:, :], PW[:, q, :], FT[:, q, c * csz:(c + 1) * csz],
                        start=(q == 0), stop=(q == cfg.fchunks - 1))
                nc.scalar.activation(
                    XLT[:, c * csz:(c + 1) * csz], pl[:, :],
                    mybir.ActivationFunctionType.Relu, bias=PB[:, :], scale=1.0)
            for j in range(NT):
                transpose_block(XC[:, j, :], XLT[:, j * 128:(j + 1) * 128], 128, 128)
            nc.gpsimd.dma_start(
                outs_d[0].ap().rearrange("(p j) f -> p (j f)", p=128), XC)

            def normalize(src, dst, q, k):
                """per-capsule l2 normalize, quarter q, (d,k)-packed rows."""
                f = k * KD
                qs, qe = q * NTh, (q + 1) * NTh
                nc.scalar.activation(SQ[:, :, :f], src[:, qs:qe, :f],
                                     mybir.ActivationFunctionType.Square)
                cur = f
                rin = RIN[:, :NTh * k]
                rin2 = RIN2[:, :NTh * k]
                while cur > k:
                    h = cur // 2
                    if h > k:
                        nc.vector.tensor_tensor(
                            out=SQ[:, :, :h], in0=SQ[:, :, :h],
                            in1=SQ[:, :, h:cur], op=mybir.AluOpType.add)
                    else:
                        nc.vector.tensor_tensor(
                            out=rin.rearrange("p (j k) -> p j k", k=k),
                            in0=SQ[:, :, :h], in1=SQ[:, :, h:cur],
                            op=mybir.AluOpType.add)
                    cur = h
                nc.vector.tensor_scalar_max(rin, rin, 1e-24)
                nc.vector.reciprocal_approx_fast(rin2, rin)
                nc.scalar.sqrt(rin2, rin2)
                if dst is not None:
                    nc.vector.tensor_tensor(
                        out=dst[:, qs:qe, :f].rearrange(
                            "p j (d k) -> p j d k", k=k),
                        in0=src[:, qs:qe, :f].rearrange(
                            "p j (d k) -> p j d k", k=k),
                        in1=rin2.rearrange("p (j k) -> p j k", k=k).unsqueeze(2)
                            .broadcast_to([128, NTh, KD, k]),
                        op=mybir.AluOpType.mult)

            # ---- routing layers -------------------------------------------
            for li, k in enumerate(cfg.caps):
                f = k * KD
                if li > 0:
                    fin = cfg.caps[li - 1] * KD
                    for j in range(NT):
                        transpose_block(XT[:fin, j * 128:(j + 1) * 128],
                                        XC[:, j, :fin], 128, fin)
                    for c in range(nchunk):
                        pl = psum.tile([128, csz], FP32, tag="pl")
                        nc.tensor.matmul(
                            pl[:f, :], WTS[li - 1][:fin, :f],
                            XT[:fin, c * csz:(c + 1) * csz],
                            start=True, stop=True)
                        nc.scalar.activation(
                            XLT[:f, c * csz:(c + 1) * csz], pl[:f, :],
                            mybir.ActivationFunctionType.Identity,
                            bias=BS[li - 1][:f, :], scale=1.0)
                    for j in range(NT):
                        transpose_block(U2[:, j, :f],
                                        XLT[:f, j * 128:(j + 1) * 128], f, 128)
                    xin = U2
                else:
                    xin = XC
                for q in range(QS):
                    normalize(xin, XN, q, k)

                nc.sync.dma_start(
                    tshard.ap().rearrange("(p j) f -> p (j f)", p=128), XN)
                nc.gpsimd.collective_compute(
                    "AllGather", mybir.AluOpType.bypass,
                    replica_groups=[list(range(cfg.n_cores))],
                    ins=[tshard.ap()], outs=[table.ap()])
                for g in range(E // GCH):
                    nc.gpsimd.dma_gather(
                        Zb[:, g * (GCH // 128):(g + 1) * (GCH // 128), :],
                        table.ap(), GIDX[:, g * (GCH // 16):(g + 1) * (GCH // 16)],
                        GCH, GCH, 128)

                B = M * NTh
                PRm = PR.rearrange("p (m j) f -> p m j f", m=M)
                for q in range(QS):
                    qs, qe = q * NTh, (q + 1) * NTh
                    zq = Zb[:, q * B:(q + 1) * B, :f]            # [128, b, f]
                    zq_dk = zq.rearrange("p b (d k) -> p b d k", k=k)
                    sq_ = S[:, :B * k].rearrange("p (b k) -> p b k", k=k)
                    sq_flat = S[:, :B * k]
                    sq4 = S[:, :B * k].rearrange(
                        "p (m j k) -> p m j k", m=M, k=k)
                    for t in range(cfg.routit):
                        usrc = XN if t == 0 else U2
                        # scores: PR = z * u ; tree-reduce over d -> S
                        nc.vector.tensor_tensor(
                            out=PRm[:, :, :, :f], in0=zq.rearrange(
                                "p (m j) f -> p m j f", m=M),
                            in1=usrc[:, qs:qe, :f].unsqueeze(1).broadcast_to(
                                [128, M, NTh, f]),
                            op=mybir.AluOpType.mult)
                        cur = f
                        while cur > k:
                            h = cur // 2
                            dst = PR[:, :, :h] if h > k else sq_
                            nc.vector.tensor_tensor(
                                out=dst, in0=PR[:, :, :h],
                                in1=PR[:, :, h:cur], op=mybir.AluOpType.add)
                            cur = h
                        if t > 0:
                            # u was left unnormalized; scale scores by 1/||u||
                            nc.vector.tensor_tensor(
                                out=sq4, in0=sq4,
                                in1=RIN2[:, :NTh * k].rearrange(
                                    "p (j k) -> p j k", k=k).unsqueeze(1)
                                    .broadcast_to([128, M, NTh, k]),
                                op=mybir.AluOpType.mult)
                        # softmax over k (scores bounded; no max shift)
                        nc.scalar.activation(sq_flat, sq_flat,
                                             mybir.ActivationFunctionType.Exp)
                        nc.vector.tensor_reduce(
                            out=DEN, in_=sq_,
                            op=mybir.AluOpType.add, axis=mybir.AxisListType.X)
                        nc.vector.reciprocal_approx_fast(RIV, DEN)
                        nc.vector.tensor_tensor(
                            out=sq_, in0=sq_,
                            in1=RIV.unsqueeze(2).broadcast_to([128, B, k]),
                            op=mybir.AluOpType.mult)
                        # aggregate: PR = z * p ; tree over m ; + x_norm
                        nc.vector.tensor_tensor(
                            out=PR[:, :, :f].rearrange(
                                "p b (d k) -> p b d k", k=k),
                            in0=zq_dk,
                            in1=sq_.unsqueeze(2).broadcast_to([128, B, KD, k]),
                            op=mybir.AluOpType.mult)
                        cm = M
                        while cm > 1:
                            h = cm // 2
                            nc.vector.tensor_tensor(
                                out=PRm[:, :h, :, :f], in0=PRm[:, :h, :, :f],
                                in1=PRm[:, h:cm, :, :f], op=mybir.AluOpType.add)
                            cm = h
                        nc.vector.tensor_tensor(
                            out=U2[:, qs:qe, :f], in0=PRm[:, 0, :, :f],
                            in1=XN[:, qs:qe, :f], op=mybir.AluOpType.add)
                        if t < cfg.routit - 1:
                            normalize(U2, None, q, k)
                        else:
                            nc.scalar.activation(
                                XC[:, qs:qe, :f], U2[:, qs:qe, :f],
                                mybir.ActivationFunctionType.Relu)
                nc.gpsimd.dma_start(
                    outs_d[li + 1].ap().rearrange("(p j) f -> p (j f)", p=128),
                    XC[:, :, :f])

    nc.compile()
    return nc


# ----------------------------------------------------------------------------
# Host-side prep / assembly
# ----------------------------------------------------------------------------

def prepare_in_maps(cfg: Cfg, feature, neighbor_id, pca_w, pca_b, ws, bs_):
    NS, NP, NT, QS, NTh = cfg.nshard, cfg.np_, cfg.nt, cfg.qs, cfg.nth
    nb = np.asarray(neighbor_id).astype(np.int64)
    perms = [perm_dk(k) for k in cfg.caps]           # routing layers 0..5
    p0 = perms[0]

    pwt = np.zeros((cfg.fpad, 128), np.float16)
    pwt[:cfg.feat, :] = np.asarray(pca_w).T[:, p0].astype(np.float16)
    pbb = np.zeros((128, 1), np.float32)
    pbb[:, 0] = np.asarray(pca_b, np.float32)[p0]
    wts, bss = [], []
    for i, (w, b) in enumerate(zip(ws, bs_)):
        fo, fi = w.shape
        wp = np.asarray(w)[perms[i + 1]][:, perms[i]]   # out-perm, in-perm
        wt = np.zeros((128, fo), np.float16)
        wt[:fi, :] = wp.T.astype(np.float16)
        wts.append(wt)
        bb = np.zeros((128, 1), np.float32)
        bb[:fo, 0] = np.asarray(b, np.float32)[perms[i + 1]]
        bss.append(bb)

    def table_row(G):
        c, n = np.divmod(G, NS)
        return c * NP + (n % 128) * NT + n // 128

    in_maps = []
    for c in range(cfg.n_cores):
        lo = c * NS
        ft = np.zeros((cfg.fpad, NP), np.float16)
        ft[:cfg.feat, :NS] = np.asarray(feature[lo:lo + NS]).T.astype(np.float16)

        rows = np.zeros((NP, M), np.int64)
        rows[:NS] = table_row(nb[lo:lo + NS, :])
        # gather order: quarter-major, then m, then node tile (j), then p
        # node n = (q*NTh + j)*128 + p ; idx position = ((q*M + m)*NTh + j)*128 + p
        r4 = rows.reshape(QS, NTh * 128, M)              # [q, n_in_q, m]
        gidx = r4.transpose(0, 2, 1).reshape(-1).astype(np.int16)
        gidx_w = np.tile(gidx.reshape(-1, 16).T, (8, 1))

        m = {"feat_t": ft, "pca_wT": pwt, "pca_b": pbb, "gidx": gidx_w}
        for i in range(len(wts)):
            m[f"w{i + 1}T"] = wts[i]
            m[f"b{i + 1}"] = bss[i]
        in_maps.append(m)
    return in_maps


def assemble_output(cfg: Cfg, results):
    NS, NT = cfg.nshard, cfg.nt
    fdims = [128] + [k * KD for k in cfg.caps]
    perms = [perm_dk(k) for k in [8] + list(cfg.caps)]
    cols = []
    for li, f in enumerate(fdims):
        perm = perms[li]
        shards = []
        for c in range(cfg.n_cores):
            a = np.asarray(results[c][f"y{li}"]).astype(np.float32)
            a = a.reshape(128, NT, f).transpose(1, 0, 2).reshape(cfg.np_, f)
            u = np.empty_like(a)
            u[:, perm] = a                                # undo (d,k) packing
            shards.append(u[:NS])
        cols.append(np.concatenate(shards, axis=0))
    return np.concatenate(cols, axis=1)


def _ensure_ntff_hook():
    try:
        from antenv.axon_hooks import get_axon_ntff_profile_hook  # noqa: F401
        return True
    except ImportError:
        pass
    try:
        import types
        import antenv
        from trn_agent_boot.trn_boot import _ntff_profile_via_ctypes
        mod = types.ModuleType("antenv.axon_hooks")
        state = {"h": None}
        mod.set_axon_ntff_profile_hook = lambda h: state.__setitem__("h", h)
        mod.get_axon_ntff_profile_hook = lambda: state["h"]
        sys.modules["antenv.axon_hooks"] = mod
        antenv.axon_hooks = mod
        mod.set_axon_ntff_profile_hook(
            _ntff_profile_via_ctypes("/opt/axon/libaxon_pjrt.so"))
        return True
    except Exception:
        return False


_CACHE = {}


def _get_nc(cfg: Cfg):
    key = (cfg.nshard, cfg.feat, cfg.n_cores)
    if key not in _CACHE:
        _CACHE[key] = build_nc(cfg)
    return _CACHE[key]


def kernel(feature, neighbor_id, pca_w, pca_b,
           w1, b1, w2, b2, w3, b3, w4, b4, w5, b5):
    cfg = FULL_CFG
    nc = _get_nc(cfg)
    in_maps = prepare_in_maps(
        cfg, np.asarray(feature), np.asarray(neighbor_id),
        np.asarray(pca_w), np.asarray(pca_b),
        [np.asarray(w) for w in (w1, w2, w3, w4, w5)],
        [np.asarray(b) for b in (b1, b2, b3, b4, b5)])
    trace = bool(int(os.environ.get("KERNEL_TRACE", "0")))
    if trace:
        trace = _ensure_ntff_hook()
    tmpdir = os.environ.get("KERNEL_TRACE_DIR") or None
    res = run_bass_kernel_spmd(nc, in_maps, core_ids=list(range(cfg.n_cores)),
                               trace=trace, tmpdir=tmpdir)
    out = assemble_output(cfg, res.results)
    if trace:
        kernel.last_exec_time_ns = res.exec_time_ns
    return out


kernel.last_exec_time_ns = None


# revision 24
# speedup vs baseline: 1.0519x; 1.0519x over previous
"""DisenGCN Trainium2 kernel (8 NeuronCores, SPMD node-parallel).

Strategy (hardcoded from the problem spec):
  - Shard the 20000 nodes across 8 cores (2500/core, padded to 2560 = 20*128).
  - Weights replicated; per layer each core computes its local normalized
    embedding shard, AllGathers the full table to DRAM, then dma_gathers its
    2560*16 neighbor rows into SBUF and runs the 5 capsule-routing iterations
    on the Vector/Scalar engines (node-major layout: nodes on partitions).
  - Features are stored in (d, k) transposed capsule order (host-side weight
    permutation) so per-capsule reductions are flat prefix-halving tree adds
    and all broadcast multiplies have contiguous innermost APs (DVE 2x mode).
  - Nodes are processed in quarters so the dma_gather descriptor generation
    (GPSIMD-bound) of quarter q+1 overlaps the routing DVE work of quarter q.
  - fp16 storage/compute; fp32 for softmax denominators / norms.
"""

import os
import sys
import numpy as np

for _p in ("/opt/trn_rl_repo", "/root/.axon_site/_ro/trn_rl_repo"):
    if os.path.isdir(_p) and _p not in sys.path:
        sys.path.insert(0, _p)

import concourse.bass as bass  # noqa: E402
import concourse.tile as tile  # noqa: E402
from concourse import bacc, mybir  # noqa: E402
from concourse.bass_utils import run_bass_kernel_spmd  # noqa: E402
from concourse.library_config import mlp as mlp_lib  # noqa: E402
from concourse.masks import make_identity  # noqa: E402

FP16 = mybir.dt.float16
FP32 = mybir.dt.float32
I16 = mybir.dt.int16

N_CORES = 8
M = 16          # neighbor fanout
KD = 16         # per-capsule dim
CAPS = [8, 7, 6, 5, 4, 3]
ROUTIT = 5
GCH = 1024      # dma_gather rows per instruction (>1024 crashes the exec unit)


def perm_dk(k):
    """new position d*k+kk  <-  old feature index kk*KD+d."""
    p = np.empty(k * KD, np.int64)
    for d in range(KD):
        for kk in range(k):
            p[d * k + kk] = kk * KD + d
    return p


class Cfg:
    def __init__(self, nshard, feat, n_cores=N_CORES, caps=CAPS, routit=ROUTIT,
                 qsplit=None):
        self.n_cores = n_cores
        self.nshard = nshard
        self.np_ = ((nshard + 127) // 128) * 128
        self.nt = self.np_ // 128
        self.E = self.np_ * M
        self.feat = feat
        self.fpad = ((feat + 127) // 128) * 128
        self.fchunks = self.fpad // 128
        self.caps = caps
        self.routit = routit
        self.kmax = max(caps)
        self.ntab = n_cores * self.np_
        if qsplit is None:
            qsplit = 4 if self.nt % 4 == 0 else 1
        self.qs = qsplit                    # node quarters
        self.nth = self.nt // qsplit        # node tiles per quarter


FULL_CFG = Cfg(2500, 500)


def build_nc(cfg: Cfg):
    nc = bacc.Bacc("TRN2", target_bir_lowering=False, debug=False,
                   num_devices=cfg.n_cores)
    NT, NP, E, QS, NTh = cfg.nt, cfg.np_, cfg.E, cfg.qs, cfg.nth
    KM = cfg.kmax

    feat_t = nc.dram_tensor("feat_t", [cfg.fpad, NP], FP16, kind="ExternalInput")
    pca_wT = nc.dram_tensor("pca_wT", [cfg.fpad, 128], FP16, kind="ExternalInput")
    pca_b = nc.dram_tensor("pca_b", [128, 1], FP32, kind="ExternalInput")
    wTs, bs = [], []
    for i in range(1, len(cfg.caps)):
        fo = cfg.caps[i] * KD
        wTs.append(nc.dram_tensor(f"w{i}T", [128, fo], FP16, kind="ExternalInput"))
        bs.append(nc.dram_tensor(f"b{i}", [128, 1], FP32, kind="ExternalInput"))
    gidx_d = nc.dram_tensor("gidx", [128, E // 16], I16, kind="ExternalInput")
    outs_d = []
    fdims = [128] + [k * KD for k in cfg.caps]
    for li, f in enumerate(fdims):
        outs_d.append(nc.dram_tensor(f"y{li}", [NP, f], FP16, kind="ExternalOutput"))

    tshard = nc.dram_tensor("tshard", [NP, 128], FP16)
    table = nc.dram_tensor("table", [cfg.ntab, 128], FP16, addr_space="Shared")

    def sb(name, shape, dt):
        return nc.alloc_sbuf_tensor(name, shape, dt).ap()

    Z = sb("Z", [128, M * NT, 128], FP16)       # [q][m][j] block order
    XC = sb("XC", [128, NT, 128], FP16)
    XN = sb("XN", [128, NT, 128], FP16)
    U = sb("U", [128, NT, 128], FP16)
    U2 = sb("U2", [128, NT, 128], FP16)
    XT = sb("XT", [128, NP], FP16)
    XLT = sb("XLT", [128, NP], FP16)
    PR = sb("PR", [128, M * NTh, 128], FP16)    # per-quarter scratch
    S = sb("S", [128, M * NTh * KM], FP16)      # scores -> exp -> p (in place)
    DEN = sb("DEN", [128, M * NTh], FP32)
    RIV = sb("RIV", [128, M * NTh], FP32)
    SQ = sb("SQ", [128, NTh, 128], FP32)        # normalize scratch
    RIN = sb("RIN", [128, NTh * KM], FP32)
    RIN2 = sb("RIN2", [128, NTh * KM], FP32)
    GIDX = sb("GIDX", [128, E // 16], I16)
    IDT = sb("IDT", [128, 128], FP16)
    FT = sb("FT", [128, cfg.fchunks, NP], FP16)
    PW = sb("PW", [128, cfg.fchunks, 128], FP16)
    PB = sb("PB", [128, 1], FP32)
    WTS = [sb(f"WTS{i}", [128, cfg.caps[i] * KD], FP16)
           for i in range(1, len(cfg.caps))]
    BS = [sb(f"BS{i}", [128, 1], FP32) for i in range(1, len(cfg.caps))]

    Zb = Z.rearrange("p (b f) -> p b f", f=128) if Z.ndim == 2 else Z
    Z5 = Z.rearrange("p (q m j) f -> p q m j f", q=QS, m=M)

    with tile.TileContext(nc) as tc:
        import contextlib
        ctx = contextlib.ExitStack()
        with ctx:
            psum = ctx.enter_context(tc.tile_pool(name="psum", bufs=2, space="PSUM"))
            nc.gpsimd.load_library(mlp_lib)
            make_identity(nc, IDT)

            nc.sync.dma_start(GIDX, gidx_d.ap())
            nc.sync.dma_start(FT, feat_t.ap().rearrange("(c p) n -> p c n", p=128))
            nc.sync.dma_start(PW, pca_wT.ap().rearrange("(c p) f -> p c f", p=128))
            nc.sync.dma_start(PB, pca_b.ap())
            for i in range(len(cfg.caps) - 1):
                nc.sync.dma_start(WTS[i], wTs[i].ap())
                nc.sync.dma_start(BS[i], bs[i].ap())

            def transpose_block(dst_ap, src_ap, fin, fout):
                pt = psum.tile([128, 128], FP16, tag="pt")
                nc.tensor.transpose(pt[:fout, :fin], src_ap, IDT[:fin, :fin])
                nc.scalar.copy(dst_ap, pt[:fout, :fin])

            # ---- PCA: XLT = relu(pca_w @ feat + b), then node-major XC ----
            nchunk = max(1, NP // 512)
            csz = NP // nchunk
            for c in range(nchunk):
                pl = psum.tile([128, csz], FP32, tag="pl")
                for q in range(cfg.fchunks):
                    nc.tensor.matmul(
                        pl[:, :], PW[:, q, :], FT[:, q, c * csz:(c + 1) * csz],
                        start=(q == 0), stop=(q == cfg.fchunks - 1))
                nc.scalar.activation(
                    XLT[:, c * csz:(c + 1) * csz], pl[:, :],
                    mybir.ActivationFunctionType.Relu, bias=PB[:, :], scale=1.0)
            for j in range(NT):
                transpose_block(XC[:, j, :], XLT[:, j * 128:(j + 1) * 128], 128, 128)
            nc.gpsimd.dma_start(
                outs_d[0].ap().rearrange("(p j) f -> p (j f)", p=128), XC)

            def normalize(src, dst, q, k):
                """per-capsule l2 normalize, quarter q, (d,k)-packed rows."""
                f = k * KD
                qs, qe = q * NTh, (q + 1) * NTh
                nc.scalar.activation(SQ[:, :, :f], src[:, qs:qe, :f],
                                     mybir.ActivationFunctionType.Square)
                cur = f
                rin = RIN[:, :NTh * k]
                rin2 = RIN2[:, :NTh * k]
                while cur > k:
                    h = cur // 2
                    if h > k:
                        nc.vector.tensor_tensor(
                            out=SQ[:, :, :h], in0=SQ[:, :, :h],
                            in1=SQ[:, :, h:cur], op=mybir.AluOpType.add)
                    else:
                        nc.vector.tensor_tensor(
                            out=rin.rearrange("p (j k) -> p j k", k=k),
                            in0=SQ[:, :, :h], in1=SQ[:, :, h:cur],
                            op=mybir.AluOpType.add)
                    cur = h
                nc.vector.tensor_scalar_max(rin, rin, 1e-24)
                nc.vector.reciprocal_approx_fast(rin2, rin)
                nc.scalar.sqrt(rin2, rin2)
                if dst is not None:
                    nc.vector.tensor_tensor(
                        out=dst[:, qs:qe, :f].rearrange(
                            "p j (d k) -> p j d k", k=k),
                        in0=src[:, qs:qe, :f].rearrange(
                            "p j (d k) -> p j d k", k=k),
                        in1=rin2.rearrange("p (j k) -> p j k", k=k).unsqueeze(2)
                            .broadcast_to([128, NTh, KD, k]),
                        op=mybir.AluOpType.mult)

            # ---- routing layers -------------------------------------------
            for li, k in enumerate(cfg.caps):
                f = k * KD
                if li > 0:
                    fin = cfg.caps[li - 1] * KD
                    for j in range(NT):
                        transpose_block(XT[:fin, j * 128:(j + 1) * 128],
                                        XC[:, j, :fin], 128, fin)
                    for c in range(nchunk):
                        pl = psum.tile([128, csz], FP32, tag="pl")
                        nc.tensor.matmul(
                            pl[:f, :], WTS[li - 1][:fin, :f],
                            XT[:fin, c * csz:(c + 1) * csz],
                            start=True, stop=True)
                        nc.scalar.activation(
                            XLT[:f, c * csz:(c + 1) * csz], pl[:f, :],
                            mybir.ActivationFunctionType.Identity,
                            bias=BS[li - 1][:f, :], scale=1.0)
                    for j in range(NT):
                        transpose_block(U2[:, j, :f],
                                        XLT[:f, j * 128:(j + 1) * 128], f, 128)
                    xin = U2
                else:
                    xin = XC
                for q in range(QS):
                    normalize(xin, XN, q, k)

                nc.sync.dma_start(
                    tshard.ap().rearrange("(p j) f -> p (j f)", p=128), XN)
                nc.gpsimd.collective_compute(
                    "AllGather", mybir.AluOpType.bypass,
                    replica_groups=[list(range(cfg.n_cores))],
                    ins=[tshard.ap()], outs=[table.ap()])
                for g in range(E // GCH):
                    nc.gpsimd.dma_gather(
                        Zb[:, g * (GCH // 128):(g + 1) * (GCH // 128), :],
                        table.ap(), GIDX[:, g * (GCH // 16):(g + 1) * (GCH // 16)],
                        GCH, GCH, 128)

                B = M * NTh
                PRm = PR.rearrange("p (m j) f -> p m j f", m=M)
                for q in range(QS):
                    qs, qe = q * NTh, (q + 1) * NTh
                    zq = Zb[:, q * B:(q + 1) * B, :f]            # [128, b, f]
                    zq_dk = zq.rearrange("p b (d k) -> p b d k", k=k)
                    sq_ = S[:, :B * k].rearrange("p (b k) -> p b k", k=k)
                    sq_flat = S[:, :B * k]
                    sq4 = S[:, :B * k].rearrange(
                        "p (m j k) -> p m j k", m=M, k=k)
                    for t in range(cfg.routit):
                        usrc = XN if t == 0 else U2
                        # scores: PR = z * u ; tree-reduce over d -> S
                        nc.vector.tensor_tensor(
                            out=PRm[:, :, :, :f], in0=zq.rearrange(
                                "p (m j) f -> p m j f", m=M),
                            in1=usrc[:, qs:qe, :f].unsqueeze(1).broadcast_to(
                                [128, M, NTh, f]),
                            op=mybir.AluOpType.mult)
                        cur = f
                        while cur > k:
                            h = cur // 2
                            dst = PR[:, :, :h] if h > k else sq_
                            nc.vector.tensor_tensor(
                                out=dst, in0=PR[:, :, :h],
                                in1=PR[:, :, h:cur], op=mybir.AluOpType.add)
                            cur = h
                        if t > 0:
                            # u was left unnormalized; scale scores by 1/||u||
                            nc.vector.tensor_tensor(
                                out=sq4, in0=sq4,
                                in1=RIN2[:, :NTh * k].rearrange(
                                    "p (j k) -> p j k", k=k).unsqueeze(1)
                                    .broadcast_to([128, M, NTh, k]),
                                op=mybir.AluOpType.mult)
                        # softmax over k (scores bounded; no max shift)
                        nc.scalar.activation(sq_flat, sq_flat,
                                             mybir.ActivationFunctionType.Exp)
                        nc.vector.tensor_reduce(
                            out=DEN, in_=sq_,
                            op=mybir.AluOpType.add, axis=mybir.AxisListType.X)
                        nc.vector.reciprocal_approx_fast(RIV, DEN)
                        nc.vector.tensor_tensor(
                            out=sq_, in0=sq_,
                            in1=RIV.unsqueeze(2).broadcast_to([128, B, k]),
                            op=mybir.AluOpType.mult)
                        # aggregate: PR = z * p ; tree over m ; + x_norm
                        nc.vector.tensor_tensor(
                            out=PR[:, :, :f].rearrange(
                                "p b (d k) -> p b d k", k=k),
                            in0=zq_dk,
                            in1=sq_.unsqueeze(2).broadcast_to([128, B, KD, k]),
                            op=mybir.AluOpType.mult)
                        cm = M
                        while cm > 1:
                            h = cm // 2
                            nc.vector.tensor_tensor(
                                out=PRm[:, :h, :, :f], in0=PRm[:, :h, :, :f],
                                in1=PRm[:, h:cm, :, :f], op=mybir.AluOpType.add)
                            cm = h
                        nc.vector.tensor_tensor(
                            out=U2[:, qs:qe, :f], in0=PRm[:, 0, :, :f],
                            in1=XN[:, qs:qe, :f], op=mybir.AluOpType.add)
                        if t < cfg.routit - 1:
                            normalize(U2, None, q, k)
                        else:
                            nc.scalar.activation(
                                XC[:, qs:qe, :f], U2[:, qs:qe, :f],
                                mybir.ActivationFunctionType.Relu)
                nc.gpsimd.dma_start(
                    outs_d[li + 1].ap().rearrange("(p j) f -> p (j f)", p=128),
                    XC[:, :, :f])

    nc.compile()
    return nc


# ----------------------------------------------------------------------------
# Host-side prep / assembly
# ----------------------------------------------------------------------------

def prepare_in_maps(cfg: Cfg, feature, neighbor_id, pca_w, pca_b, ws, bs_):
    NS, NP, NT, QS, NTh = cfg.nshard, cfg.np_, cfg.nt, cfg.qs, cfg.nth
    nb = np.asarray(neighbor_id).astype(np.int64)
    perms = [perm_dk(k) for k in cfg.caps]           # routing layers 0..5
    p0 = perms[0]

    pwt = np.zeros((cfg.fpad, 128), np.float16)
    pwt[:cfg.feat, :] = np.asarray(pca_w).T[:, p0].astype(np.float16)
    pbb = np.zeros((128, 1), np.float32)
    pbb[:, 0] = np.asarray(pca_b, np.float32)[p0]
    wts, bss = [], []
    for i, (w, b) in enumerate(zip(ws, bs_)):
        fo, fi = w.shape
        wp = np.asarray(w)[perms[i + 1]][:, perms[i]]   # out-perm, in-perm
        wt = np.zeros((128, fo), np.float16)
        wt[:fi, :] = wp.T.astype(np.float16)
        wts.append(wt)
        bb = np.zeros((128, 1), np.float32)
        bb[:fo, 0] = np.asarray(b, np.float32)[perms[i + 1]]
        bss.append(bb)

    def table_row(G):
        c, n = np.divmod(G, NS)
        return c * NP + (n % 128) * NT + n // 128

    in_maps = []
    for c in range(cfg.n_cores):
        lo = c * NS
        ft = np.zeros((cfg.fpad, NP), np.float16)
        ft[:cfg.feat, :NS] = np.asarray(feature[lo:lo + NS]).T.astype(np.float16)

        rows = np.zeros((NP, M), np.int64)
        rows[:NS] = table_row(nb[lo:lo + NS, :])
        # gather order: quarter-major, then m, then node tile (j), then p
        # node n = (q*NTh + j)*128 + p ; idx position = ((q*M + m)*NTh + j)*128 + p
        r4 = rows.reshape(QS, NTh * 128, M)              # [q, n_in_q, m]
        gidx = r4.transpose(0, 2, 1).reshape(-1).astype(np.int16)
        gidx_w = np.tile(gidx.reshape(-1, 16).T, (8, 1))

        m = {"feat_t": ft, "pca_wT": pwt, "pca_b": pbb, "gidx": gidx_w}
        for i in range(len(wts)):
            m[f"w{i + 1}T"] = wts[i]
            m[f"b{i + 1}"] = bss[i]
        in_maps.append(m)
    return in_maps


def assemble_output(cfg: Cfg, results):
    NS, NT = cfg.nshard, cfg.nt
    fdims = [128] + [k * KD for k in cfg.caps]
    perms = [perm_dk(k) for k in [8] + list(cfg.caps)]
    cols = []
    for li, f in enumerate(fdims):
        perm = perms[li]
        shards = []
        for c in range(cfg.n_cores):
            a = np.asarray(results[c][f"y{li}"]).astype(np.float32)
            a = a.reshape(128, NT, f).transpose(1, 0, 2).reshape(cfg.np_, f)
            u = np.empty_like(a)
            u[:, perm] = a                                # undo (d,k) packing
            shards.append(u[:NS])
        cols.append(np.concatenate(shards, axis=0))
    return np.concatenate(cols, axis=1)


def _ensure_ntff_hook():
    try:
        from antenv.axon_hooks import get_axon_ntff_profile_hook  # noqa: F401
        return True
    except ImportError:
        pass
    try:
        import types
        import antenv
        from trn_agent_boot.trn_boot import _ntff_profile_via_ctypes
        mod = types.ModuleType("antenv.axon_hooks")
        state = {"h": None}
        mod.set_axon_ntff_profile_hook = lambda h: state.__setitem__("h", h)
        mod.get_axon_ntff_profile_hook = lambda: state["h"]
        sys.modules["antenv.axon_hooks"] = mod
        antenv.axon_hooks = mod
        mod.set_axon_ntff_profile_hook(
            _ntff_profile_via_ctypes("/opt/axon/libaxon_pjrt.so"))
        return True
    except Exception:
        return False


_CACHE = {}


def _get_nc(cfg: Cfg):
    key = (cfg.nshard, cfg.feat, cfg.n_cores)
    if key not in _CACHE:
        _CACHE[key] = build_nc(cfg)
    return _CACHE[key]


def kernel(feature, neighbor_id, pca_w, pca_b,
           w1, b1, w2, b2, w3, b3, w4, b4, w5, b5):
    cfg = FULL_CFG
    nc = _get_nc(cfg)
    in_maps = prepare_in_maps(
        cfg, np.asarray(feature), np.asarray(neighbor_id),
        np.asarray(pca_w), np.asarray(pca_b),
        [np.asarray(w) for w in (w1, w2, w3, w4, w5)],
        [np.asarray(b) for b in (b1, b2, b3, b4, b5)])
    trace = bool(int(os.environ.get("KERNEL_TRACE", "0")))
    if trace:
        trace = _ensure_ntff_hook()
    tmpdir = os.environ.get("KERNEL_TRACE_DIR") or None
    res = run_bass_kernel_spmd(nc, in_maps, core_ids=list(range(cfg.n_cores)),
                               trace=trace, tmpdir=tmpdir)
    out = assemble_output(cfg, res.results)
    if trace:
        kernel.last_exec_time_ns = res.exec_time_ns
    return out


kernel.last_exec_time_ns = None
